# revision 2
# baseline (speedup 1.0000x reference)
"""ChebConv (K=4) GNN message passing on 8 Trainium2 NeuronCores.

Strategy (1D vertex partitioning, hardcoded for N=100000, E=1600000,
D_in=D_out=64, K=4, lambda_max=2.0):
  - Nodes are sharded contiguously: core c owns rows [c*12544, (c+1)*12544)
    of a zero-padded 100352-row node table (only core 7's tail is padding).
  - Edges are partitioned by dst owner. Each core sorts its edges by
    (src_group, dst) where src_group = src // 25088 (4 groups so local
    gather indices fit in int16 for dma_gather), padding each
    (group, 128-dst-block) run to a multiple of 128 edge slots.
  - Per propagation: every core computes its shard of x' = Tx * norm,
    AllGathers the full x' table into local DRAM, fetches per-edge source
    rows with chunked dma_gather calls (<=4096 idxs each; larger calls hang
    the SWDGE), and segment-sums each 128-edge tile with a one-hot
    selection matmul (sel[e,d] = (local_dst[e]==d)) accumulating
    per-dst-block in PSUM; PSUM drains add into the new-Tx accumulator.
    The Chebyshev recurrence (lambda_max=2: Tx1 = -h, Tx_i = -2h - Tx_{i-2})
    is applied per block after all 4 group passes.
  - rst = sum_i Tx_i @ W_i + b is computed per 128-node block with PE
    transposes and W-stationary matmuls; the per-core output is
    feature-major [64, 12544] and the host transposes/concatenates.

The degree array (edge counts per dst) is a byproduct of the host-side
edge partitioning; norm = 1/sqrt(max(deg,1)) is computed on device.
"""
import numpy as np
from contextlib import ExitStack

import concourse.bass as bass
import concourse.tile as tile
from concourse import bacc, mybir
from concourse.bass_utils import run_bass_kernel_spmd
from concourse.masks import make_identity

N = 100000
E = 1600000
D = 64
KORD = 4
NCORES = 8
SHARD = 12544
NPAD = SHARD * NCORES   # 100352
NBLK = SHARD // 128     # 98
P = 128
NGRP = 4
GRP = NPAD // NGRP      # 25088 rows per gather group (int16-safe)
CHUNK_TILES = 32        # 4096 idxs per dma_gather call

PAD_SELCOL = 999.0


def _prepare_edges(src: np.ndarray, dst: np.ndarray):
    """Partition + sort edges per core; build idx16/selcol streams.

    Slot layout: for g in 0..3: for b in 0..97: tiles_per[g][b] tiles of 128
    slots. Streams (identical shapes on every core):
      idx16 [128, NSLOT//16] int16, slot i at [i%16, i//16], replicated x8
      selcol [128, NTILE] f32, slot of tile t, lane p at [p, t]
    """
    per_core = []
    counts = np.zeros((NCORES, NGRP, NBLK), dtype=np.int64)
    for c in range(NCORES):
        lo, hi = c * SHARD, min((c + 1) * SHARD, N)
        m = (dst >= lo) & (dst < hi)
        s, d = src[m], dst[m] - lo
        g = s // GRP
        b = d >> 7
        order = np.lexsort((d, b, g))
        s, d, g, b = s[order], d[order], g[order], b[order]
        np.add.at(counts[c], (g, b), 1)
        per_core.append((s, d, g, b))
    tiles_per = np.maximum(1, (counts.max(axis=0) + 127) // 128)  # [NGRP, NBLK]
    ntile = int(tiles_per.sum())
    nslot = ntile * P
    tile_start = np.zeros((NGRP, NBLK), dtype=np.int64)
    acc = 0
    for g in range(NGRP):
        for b in range(NBLK):
            tile_start[g, b] = acc
            acc += tiles_per[g, b]

    idx_streams, sel_streams = [], []
    for c in range(NCORES):
        s, d, g, b = per_core[c]
        flat_idx = np.zeros(nslot, dtype=np.int16)
        flat_sel = np.full(nslot, PAD_SELCOL, dtype=np.float32)
        cnt = counts[c]
        estart = np.concatenate([[0], np.cumsum(cnt.ravel())])
        for gg in range(NGRP):
            for bb in range(NBLK):
                k = gg * NBLK + bb
                e0, e1 = estart[k], estart[k + 1]
                if e1 == e0:
                    continue
                s0 = tile_start[gg, bb] * P
                n = e1 - e0
                flat_idx[s0:s0 + n] = (s[e0:e1] - gg * GRP).astype(np.int16)
                flat_sel[s0:s0 + n] = (d[e0:e1] & 127).astype(np.float32)
        idx16 = np.tile(flat_idx.reshape(nslot // 16, 16).T, (8, 1))
        selcol = np.ascontiguousarray(flat_sel.reshape(ntile, P).T)
        idx_streams.append(np.ascontiguousarray(idx16))
        sel_streams.append(selcol)
    return tiles_per, tile_start, ntile, idx_streams, sel_streams


def _build_nc(tiles_per, tile_start, ntile, debug_taps=False):
    nc = bacc.Bacc("TRN2", target_bir_lowering=False, debug=False,
                   enable_asserts=True, num_devices=NCORES)
    f32 = mybir.dt.float32
    nslot16 = ntile * P // 16

    feat_in = nc.dram_tensor("feat", [SHARD, D], f32, kind="ExternalInput").ap()
    deg_in = nc.dram_tensor("deg", [P, NBLK], f32, kind="ExternalInput").ap()
    idx_in = nc.dram_tensor("idx", [P, nslot16], mybir.dt.int16, kind="ExternalInput").ap()
    sel_in = nc.dram_tensor("selcol", [P, ntile], f32, kind="ExternalInput").ap()
    iota_in = nc.dram_tensor("iota", [P, P], f32, kind="ExternalInput").ap()
    w_in = nc.dram_tensor("w", [D, KORD * D], f32, kind="ExternalInput").ap()
    b_in = nc.dram_tensor("bias", [D, 1], f32, kind="ExternalInput").ap()
    out = nc.dram_tensor("rstT", [D, SHARD], f32, kind="ExternalOutput").ap()
    if debug_taps:
        dbg_tx1 = nc.dram_tensor("dbg_tx1", [P, NBLK * D], f32, kind="ExternalOutput").ap()

    with tile.TileContext(nc) as tc:
        with ExitStack() as ctx:
            const = ctx.enter_context(tc.tile_pool(name="const", bufs=1))
            txp = ctx.enter_context(tc.tile_pool(name="txp", bufs=1))
            msgp = ctx.enter_context(tc.tile_pool(name="msgp", bufs=3))
            selp = ctx.enter_context(tc.tile_pool(name="selp", bufs=3))
            psum = ctx.enter_context(tc.tile_pool(name="psum", bufs=4, space="PSUM"))
            trp = ctx.enter_context(tc.tile_pool(name="trp", bufs=2, space="PSUM"))
            rstp = ctx.enter_context(tc.tile_pool(name="rstp", bufs=2, space="PSUM"))
            outp = ctx.enter_context(tc.tile_pool(name="outp", bufs=3))
            featp = ctx.enter_context(tc.tile_pool(name="featp", bufs=2))
            dram = ctx.enter_context(tc.tile_pool(name="dram", bufs=1, space="DRAM"))

            # ---- constants / streams
            idx_sb = const.tile([P, nslot16], mybir.dt.int16)
            nc.sync.dma_start(idx_sb[:], idx_in[:])
            sel_sb = const.tile([P, ntile], f32)
            nc.sync.dma_start(sel_sb[:], sel_in[:])
            iota_sb = const.tile([P, P], f32)
            nc.sync.dma_start(iota_sb[:], iota_in[:])
            w_sb = const.tile([D, KORD * D], f32)
            nc.sync.dma_start(w_sb[:], w_in[:])
            b_sb = const.tile([D, 1], f32)
            nc.sync.dma_start(b_sb[:], b_in[:])
            ident = const.tile([P, P], f32)
            make_identity(nc, ident[:])

            # ---- norm = 1/sqrt(max(deg,1)); nnorm = -norm; n2norm = -2*norm
            deg_sb = const.tile([P, NBLK], f32)
            nc.sync.dma_start(deg_sb[:], deg_in[:])
            norm = const.tile([P, NBLK], f32)
            nc.vector.tensor_scalar_max(norm[:], deg_sb[:], 1.0)
            nc.scalar.activation(norm[:], norm[:], mybir.ActivationFunctionType.Sqrt)
            nc.vector.reciprocal(norm[:], norm[:])
            nnorm = const.tile([P, NBLK], f32)
            nc.vector.tensor_scalar_mul(nnorm[:], norm[:], -1.0)
            n2norm = const.tile([P, NBLK], f32)
            nc.vector.tensor_scalar_mul(n2norm[:], norm[:], -2.0)

            # ---- Tx ring buffers (node-major [p, b*64+f])
            txA = txp.tile([P, NBLK * D], f32)   # feat -> later Tx3
            txB = txp.tile([P, NBLK * D], f32)   # Tx1
            txC = txp.tile([P, NBLK * D], f32)   # Tx2
            xn = txp.tile([P, NBLK * D], f32)
            nc.sync.dma_start(
                txA[:].rearrange("p (b f) -> p b f", b=NBLK),
                feat_in.rearrange("(b p) f -> p b f", p=P))

            table_own = dram.tile([SHARD, D], f32, name="table_own")
            table_full = [
                dram.tile([NPAD, D], f32, addr_space="Shared", name=f"tfull{p}")
                for p in range(KORD - 1)
            ]
            tx_ring = [txA, txB, txC]

            for prop in range(1, KORD):
                tx_cur = tx_ring[(prop - 1) % 3]
                tx_new = tx_ring[prop % 3]
                tx_prev2 = tx_ring[(prop - 2) % 3]

                # xn = tx_cur * norm
                for b in range(NBLK):
                    nc.vector.tensor_tensor(
                        out=xn[:, b * D:(b + 1) * D],
                        in0=tx_cur[:, b * D:(b + 1) * D],
                        in1=norm[:, b:b + 1].to_broadcast([P, D]),
                        op=mybir.AluOpType.mult,
                    )
                nc.sync.dma_start(
                    table_own.opt().rearrange("(b p) f -> p b f", p=P),
                    xn[:].rearrange("p (b f) -> p b f", b=NBLK))
                nc.gpsimd.collective_compute(
                    "AllGather", mybir.AluOpType.bypass,
                    replica_groups=[list(range(NCORES))],
                    ins=[table_own.opt()],
                    outs=[table_full[prop - 1].opt()],
                )
                tbl = table_full[prop - 1].opt()

                # chunked gathers (per group) feeding per-(group,block) matmuls
                chunk_tiles = {}
                for g in range(NGRP):
                    gt0 = int(tile_start[g, 0])
                    gt1 = int(tile_start[g, NBLK - 1] + tiles_per[g, NBLK - 1])
                    j = gt0
                    ci = 0
                    while j < gt1:
                        cnt = min(CHUNK_TILES, gt1 - j)
                        m = msgp.tile([P, CHUNK_TILES, D], f32, tag="msg",
                                      name=f"m{prop}_{g}_{ci}")
                        nc.gpsimd.dma_gather(
                            out_ap=m[:, 0:cnt, :],
                            in_ap=tbl[g * GRP:(g + 1) * GRP, :],
                            idxs_ap=idx_sb[:, j * 8:(j + cnt) * 8],
                            num_idxs=cnt * P,
                            num_idxs_reg=cnt * P,
                            elem_size=D,
                            single_packet=False,
                        )
                        for jl in range(cnt):
                            chunk_tiles[j + jl] = (m, jl)
                        j += cnt
                        ci += 1

                for g in range(NGRP):
                    for b in range(NBLK):
                        tb = int(tiles_per[g, b])
                        j0 = int(tile_start[g, b])
                        ps = psum.tile([P, D], f32, tag="ps", name=f"ps{prop}_{g}_{b}")
                        for t in range(tb):
                            jj = j0 + t
                            m, jl = chunk_tiles[jj]
                            sel = selp.tile([P, P], f32, tag="sel",
                                            name=f"sel{prop}_{g}_{b}_{t}")
                            nc.vector.tensor_tensor(
                                out=sel[:],
                                in0=sel_sb[:, jj:jj + 1].to_broadcast([P, P]),
                                in1=iota_sb[:],
                                op=mybir.AluOpType.is_equal,
                            )
                            nc.tensor.matmul(
                                ps[:], lhsT=sel[:], rhs=m[:, jl, :],
                                start=(t == 0), stop=(t == tb - 1),
                            )
                        dst_sl = tx_new[:, b * D:(b + 1) * D]
                        if g == 0:
                            nc.vector.tensor_copy(dst_sl, ps[:])
                        else:
                            nc.vector.tensor_tensor(
                                out=dst_sl, in0=dst_sl, in1=ps[:],
                                op=mybir.AluOpType.add)

                # recurrence: Tx1 = -h*norm ; Tx_i = -2*h*norm - Tx_{i-2}
                for b in range(NBLK):
                    dst_sl = tx_new[:, b * D:(b + 1) * D]
                    scale = nnorm if prop == 1 else n2norm
                    nc.vector.tensor_tensor(
                        out=dst_sl, in0=dst_sl,
                        in1=scale[:, b:b + 1].to_broadcast([P, D]),
                        op=mybir.AluOpType.mult)
                    if prop >= 2:
                        nc.vector.tensor_tensor(
                            out=dst_sl, in0=dst_sl,
                            in1=tx_prev2[:, b * D:(b + 1) * D],
                            op=mybir.AluOpType.subtract)

            if debug_taps:
                nc.sync.dma_start(dbg_tx1[:], txB[:])

            # ---- rst = sum_i Tx_i @ W_i + b  (feature-major per block)
            for b in range(NBLK):
                rst_ps = rstp.tile([D, P], f32, tag="rst", name=f"rst{b}")
                featb = featp.tile([P, D], f32, tag="fb", name=f"fb{b}")
                nc.sync.dma_start(featb[:], feat_in[b * P:(b + 1) * P, :])
                srcs = [featb[:, :], txB[:, b * D:(b + 1) * D],
                        txC[:, b * D:(b + 1) * D], txA[:, b * D:(b + 1) * D]]
                txT = outp.tile([D, KORD * P], f32, tag="txT", name=f"txT{b}")
                for i in range(KORD):
                    trp_ps = trp.tile([D, P], f32, tag="tr", name=f"tr{b}_{i}")
                    nc.tensor.transpose(trp_ps[:], srcs[i], ident[:])
                    nc.vector.tensor_copy(txT[:, i * P:(i + 1) * P], trp_ps[:])
                for i in range(KORD):
                    nc.tensor.matmul(
                        rst_ps[:], lhsT=w_sb[:, i * D:(i + 1) * D],
                        rhs=txT[:, i * P:(i + 1) * P],
                        start=(i == 0), stop=(i == KORD - 1),
                    )
                ostage = outp.tile([D, P], f32, tag="ostage", name=f"os{b}")
                nc.vector.tensor_tensor(
                    out=ostage[:], in0=rst_ps[:],
                    in1=b_sb[:, 0:1].to_broadcast([D, P]),
                    op=mybir.AluOpType.add)
                nc.sync.dma_start(out[:, b * P:(b + 1) * P], ostage[:])
    nc.compile()
    return nc


_CACHE = {}


def _get_compiled(src: np.ndarray, dst: np.ndarray, debug_taps=False):
    key = (src.tobytes()[:256], dst.tobytes()[:256], len(src), debug_taps)
    if key not in _CACHE:
        tpb, ts, ntile, idx_s, sel_s = _prepare_edges(src, dst)
        nc = _build_nc(tpb, ts, ntile, debug_taps=debug_taps)
        _CACHE[key] = (nc, idx_s, sel_s)
    return _CACHE[key]


def _make_in_maps(feat, src, dst, W, b, idx_s, sel_s):
    deg = np.bincount(dst, minlength=NPAD).astype(np.float32)
    feat_pad = np.zeros((NPAD, D), dtype=np.float32)
    feat_pad[:N] = feat
    iota = np.broadcast_to(np.arange(P, dtype=np.float32)[None, :], (P, P)).copy()
    w_flat = np.ascontiguousarray(W.astype(np.float32).transpose(1, 0, 2).reshape(D, KORD * D))
    b_col = np.ascontiguousarray(b.astype(np.float32).reshape(D, 1))
    in_maps = []
    for c in range(NCORES):
        sl = slice(c * SHARD, (c + 1) * SHARD)
        in_maps.append({
            "feat": feat_pad[sl],
            "deg": np.ascontiguousarray(deg[sl].reshape(NBLK, P).T),
            "idx": idx_s[c],
            "selcol": sel_s[c],
            "iota": iota,
            "w": w_flat,
            "bias": b_col,
        })
    return in_maps


def _unshard(parts):
    return np.concatenate(parts, axis=0)[:N].astype(np.float32)


def kernel(feat, src, dst, W, b):
    nc, idx_s, sel_s = _get_compiled(src, dst)
    in_maps = _make_in_maps(feat, src, dst, W, b, idx_s, sel_s)
    res = run_bass_kernel_spmd(nc, in_maps, list(range(NCORES)))
    parts = [res.results[c]["rstT"].T for c in range(NCORES)]
    return _unshard(parts)

